# revision 1
# baseline (speedup 1.0000x reference)
"""Trainium2 Bass kernel for a BasicTransformerBlock (self-attn + cross-attn + GEGLU FF).

Contract: kernel(**inputs) takes FULL unsharded inputs (np arrays keyed as in
setup_inputs()) and returns the FULL [8, 1024, 512] float32 output.

Sharding: data-parallel over batch B=8 across 8 NeuronCores (one batch element
per core, all weights replicated, no collectives).
"""

import os

import numpy as np

import concourse.bass as bass
import concourse.tile as tile
from concourse import mybir
from concourse.bass_utils import run_bass_kernel_spmd
from concourse.masks import make_identity

# ---------------------------------------------------------------------------
# Workaround: this toolchain's walrus encodes at most ONE sync-wait per
# instruction (2 for EventSemaphore). Tile attaches one wait per producer
# proc, so after scheduling we hoist excess waits onto prepended same-engine
# NOPs -- semantically identical (the engine blocks at the NOP instead).
# ---------------------------------------------------------------------------
def _legalize_wait_counts(nc, max_waits=1):
    n_moved = 0
    for f in nc.m.functions:
        for bb in f.blocks:
            out, changed = [], False
            for inst in bb.instructions:
                si = inst.sync_info
                waits = list(si.on_wait) if si is not None and si.on_wait else []
                cap = 2 if isinstance(inst, mybir.InstEventSemaphore) else max_waits
                if len(waits) > cap:
                    keep, rest = waits[:cap], waits[cap:]
                    for i in range(0, len(rest), max_waits):
                        out.append(mybir.InstNoOp(
                            name=f"{inst.name}-lw{i}",
                            engine=inst.engine,
                            bass_nofuse=True,
                            sync_info=mybir.SyncInfo(
                                on_wait=rest[i:i + max_waits], on_update=[]),
                        ))
                    si.on_wait = keep
                    inst.sync_info = si
                    n_moved += len(rest)
                    changed = True
                out.append(inst)
            if changed:
                bb.instructions = out
    return n_moved


# ---------------------------------------------------------------------------
# Problem shapes (hardcoded per contract)
# ---------------------------------------------------------------------------
P = 128
B, L, D, S, CD, H, DH = 8, 1024, 512, 77, 768, 8, 64
FF = 2048                 # GEGLU inner dim; ff_w1 is [D, 2*FF]
LT = L // P               # 8 token tiles
KC = D // P               # 4 channel chunks of the model dim
CC = CD // P              # 6 context channel chunks
NH = 512                  # moving-operand chunk (PSUM bank = 512 fp32)
TH = L // NH              # 2 token halves
FC = FF // P              # 16 chunks of the FF inner dim
EPS = 1e-5
NCORES = 8

F32 = mybir.dt.float32
F32R = mybir.dt.float32r
BF16 = mybir.dt.bfloat16
AF = mybir.ActivationFunctionType
ALU = mybir.AluOpType


def _build_nc():
    nc = bass.Bass(target_bir_lowering=False, debug=True)

    pr = {}
    pr["x"] = nc.declare_dram_parameter("x", [L, D], F32, isOutput=False)
    pr["context"] = nc.declare_dram_parameter("context", [S, CD], F32, isOutput=False)
    for nm in ("ln1_g", "ln1_b", "ln2_g", "ln2_b", "ln3_g", "ln3_b"):
        pr[nm] = nc.declare_dram_parameter(nm, [D], F32, isOutput=False)
    for nm in ("a1_bo", "a2_bo", "ff_b2"):
        # consumed (only) as fp32r matmul operands via the rank-1 bias trick
        pr[nm] = nc.declare_dram_parameter(nm, [D], F32R, isOutput=False)
    for nm in ("a1_wq", "a1_wk", "a1_wv", "a1_wo", "a2_wq", "a2_wo"):
        pr[nm] = nc.declare_dram_parameter(nm, [D, D], F32R, isOutput=False)
    for nm in ("a2_wk", "a2_wv"):
        pr[nm] = nc.declare_dram_parameter(nm, [CD, D], F32, isOutput=False)
    pr["ff_w1"] = nc.declare_dram_parameter("ff_w1", [D, 2 * FF], F32R, isOutput=False)
    pr["ff_b1"] = nc.declare_dram_parameter("ff_b1", [2 * FF], F32, isOutput=False)
    pr["ff_w2"] = nc.declare_dram_parameter("ff_w2", [FF, D], F32, isOutput=False)
    out_p = nc.declare_dram_parameter("out", [L, D], F32, isOutput=True)

    reps = int(os.environ.get("BASS_KERNEL_REPS", "1"))
    with tile.TileContext(nc) as tc, \
         nc.allow_low_precision(reason="fp32r tiles feed fp32r matmuls; all "
                                       "matmul accumulation stays fp32 in PSUM"):
        if reps > 1:
            with tc.For_i(0, reps, 1):
                _emit(nc, tc, pr, out_p)
        else:
            _emit(nc, tc, pr, out_p)
    if os.environ.get("BASS_KERNEL_SKIP_WAIT_LEGALIZE") != "1":
        _legalize_wait_counts(nc)
    return nc


def _emit(nc, tc, pr, out_p):
    from contextlib import ExitStack

    top = ExitStack()
    with top:
        # ------------------ persistent pools (whole kernel) -----------------
        const = top.enter_context(tc.tile_pool(name="const", bufs=1))
        xpool = top.enter_context(tc.tile_pool(name="xsb", bufs=1))
        htpool = top.enter_context(tc.tile_pool(name="hT", bufs=1))
        lntp = top.enter_context(tc.tile_pool(name="lnt", bufs=3))
        qkpool = top.enter_context(tc.tile_pool(name="qkT", bufs=1))
        opool = top.enter_context(tc.tile_pool(name="oT", bufs=1))

        # ----------------------------- constants ---------------------------
        identity = const.tile([P, P], F32, tag="identity", name="identity")
        make_identity(nc, identity[:, :])
        ones_f = const.tile([1, P], F32, tag="ones_f", name="ones_f")
        nc.vector.memset(ones_f[:, :], 1.0)
        ones = const.tile([1, P], F32R, tag="ones", name="ones")
        nc.vector.tensor_copy(ones[:, :], ones_f[:, :])
        eps_t = const.tile([P, 1], F32, tag="eps", name="eps")
        nc.vector.memset(eps_t[:, :], EPS)

        # ------------------------------ DMAs in -----------------------------
        xsb = []
        for t in range(LT):
            xt = xpool.tile([P, D], F32, tag=f"x{t}", name=f"x{t}")
            nc.sync.dma_start(out=xt[:, :], in_=pr["x"][t * P:(t + 1) * P, :])
            xsb.append(xt)

        def load_w(pool, nm, rows, dst_dtype=F32, stage=None):
            """Load a [rows, 512] weight as row-chunk tiles; optional bf16
            cast staged through `stage` pool on the (otherwise idle) GpSimd."""
            tiles = []
            for c in range(rows // P):
                if stage is None:
                    w = pool.tile([P, D], dst_dtype, tag=f"{nm}{c}", name=f"{nm}{c}")
                    nc.sync.dma_start(out=w[:, :], in_=pr[nm][c * P:(c + 1) * P, :])
                else:
                    wf = stage.tile([P, D], F32, tag="stage", name=f"stg_{nm}{c}")
                    nc.sync.dma_start(out=wf[:, :], in_=pr[nm][c * P:(c + 1) * P, :])
                    w = pool.tile([P, D], dst_dtype, tag=f"{nm}{c}", name=f"{nm}{c}")
                    nc.gpsimd.tensor_copy(w[:, :], wf[:, :])
                tiles.append(w)
            return tiles

        es_a1 = ExitStack()
        a1pool = es_a1.enter_context(tc.tile_pool(name="a1w", bufs=1))
        wq1 = load_w(a1pool, "a1_wq", D, dst_dtype=F32R)
        wk1 = load_w(a1pool, "a1_wk", D, dst_dtype=F32R)
        wv1 = load_w(a1pool, "a1_wv", D, dst_dtype=F32R)
        wo1 = load_w(a1pool, "a1_wo", D, dst_dtype=F32R)

        # LN gamma/beta: six [512] vectors stacked as [4,128] rows -> [24,128],
        # one transpose -> gb [128,24]; ff_b1 [4096] -> [32,128] -> fb1 [128,32]
        lnstack = const.tile([P, P], F32, tag="lnstack", name="lnstack")
        for i, (gnm, bnm) in enumerate((("ln1_g", "ln1_b"), ("ln2_g", "ln2_b"),
                                        ("ln3_g", "ln3_b"))):
            nc.sync.dma_start(out=lnstack[i * 8:i * 8 + 4, :],
                              in_=pr[gnm].rearrange("(a f) -> a f", f=P))
            nc.sync.dma_start(out=lnstack[i * 8 + 4:i * 8 + 8, :],
                              in_=pr[bnm].rearrange("(a f) -> a f", f=P))
        fb1stack = const.tile([32, P], F32, tag="fb1stack", name="fb1stack")
        nc.sync.dma_start(out=fb1stack[:, :],
                          in_=pr["ff_b1"].rearrange("(a f) -> a f", f=P))
        # bias rows for the rank-1 PSUM-preload trick
        bo1_row = const.tile([1, D], F32R, tag="bo1row", name="bo1row")
        nc.sync.dma_start(out=bo1_row[:, :],
                          in_=pr["a1_bo"].rearrange("(o f) -> o f", o=1))
        bo2_row = const.tile([1, D], F32R, tag="bo2row", name="bo2row")
        nc.sync.dma_start(out=bo2_row[:, :],
                          in_=pr["a2_bo"].rearrange("(o f) -> o f", o=1))
        fb2_row = const.tile([1, D], F32R, tag="fb2row", name="fb2row")
        nc.sync.dma_start(out=fb2_row[:, :],
                          in_=pr["ff_b2"].rearrange("(o f) -> o f", o=1))

        gb = const.tile([P, 24], F32, tag="gb", name="gb")
        fb1 = const.tile([P, 32], F32, tag="fb1", name="fb1")
        with tc.tile_pool(name="psprep", bufs=2, space="PSUM") as psprep:
            pst = psprep.tile([P, P], F32, tag="pstr", name="pstr_gb")
            nc.tensor.transpose(pst[:, 0:24], lnstack[0:24, :], identity[0:24, 0:24])
            nc.vector.tensor_copy(gb[:, :], pst[:, 0:24])
            pst2 = psprep.tile([P, P], F32, tag="pstr", name="pstr_fb1")
            nc.tensor.transpose(pst2[:, 0:32], fb1stack[0:32, :], identity[0:32, 0:32])
            nc.vector.tensor_copy(fb1[:, :], pst2[:, 0:32])

        def g_col(i, kc):
            return gb[:, i * 8 + kc:i * 8 + kc + 1]

        def b_col(i, kc):
            return gb[:, i * 8 + 4 + kc:i * 8 + 4 + kc + 1]

        # ------------------------------------------------------------------
        # LayerNorm -> feature-major hT [128ch, 1024tok] x 4 (tags shared
        # across sections; gamma/beta folded into the transpose eviction)
        # ------------------------------------------------------------------
        def layer_norm_T(ln_i):
            hT = [htpool.tile([P, L], F32R, tag=f"hT{c}", name=f"hT{ln_i}_{c}")
                  for c in range(KC)]
            with tc.tile_pool(name=f"lnps{ln_i}", bufs=3, space="PSUM") as lnps:
                for t in range(LT):
                    st = lntp.tile([P, 6], F32, tag="bnstats", name="bnstats")
                    nc.vector.bn_stats(st[:, :], xsb[t][:, :])
                    mv = lntp.tile([P, 2], F32, tag="bnaggr", name="bnaggr")
                    nc.vector.bn_aggr(mv[:, :], st[:, :])
                    # rstd = exp(-0.5*ln(var+eps)); Ln/Exp share one ACT table
                    # set with Identity/Copy (Sqrt would force a set switch).
                    lnv = lntp.tile([P, 1], F32, tag="lnv", name="lnv")
                    nc.scalar.activation(lnv[:, :], mv[:, 1:2], AF.Ln,
                                         bias=eps_t[:, :])
                    rstd = lntp.tile([P, 1], F32, tag="rstd", name="rstd")
                    nc.scalar.activation(rstd[:, :], lnv[:, :], AF.Exp, scale=-0.5)
                    nmr = lntp.tile([P, 1], F32, tag="nmr", name="nmr")
                    nc.vector.tensor_scalar(nmr[:, :], mv[:, 0:1], rstd[:, :], -1.0,
                                            op0=ALU.mult, op1=ALU.mult)
                    z = lntp.tile([P, D], F32, tag="z", name="z")
                    nc.scalar.activation(z[:, :], xsb[t][:, :], AF.Identity,
                                         bias=nmr[:, :], scale=rstd[:, :])
                    for c in range(KC):
                        ps = lnps.tile([P, P], F32, tag="ps", name="lntr")
                        nc.tensor.transpose(ps[:, :], z[:, c * P:(c + 1) * P],
                                            identity[:, :])
                        nc.vector.tensor_scalar(hT[c][:, t * P:(t + 1) * P],
                                                ps[:, :],
                                                g_col(ln_i, c), b_col(ln_i, c),
                                                op0=ALU.mult, op1=ALU.add)
            return hT

        # feature-major projection (weights stationary), bf16 output
        def proj_featT(w_tiles, hT, pspool, out_tag, scale=None):
            outT = []
            for oc in range(KC):
                ot = qkpool.tile([P, L], BF16, tag=f"{out_tag}{oc}",
                                 name=f"{out_tag}{oc}")
                for th in range(TH):
                    ps = pspool.tile([P, NH], F32, tag="ps", name=f"ps_{out_tag}")
                    for kc in range(KC):
                        nc.tensor.matmul(
                            ps[:, :],
                            lhsT=w_tiles[kc][:, oc * P:(oc + 1) * P],
                            rhs=hT[kc][:, th * NH:(th + 1) * NH],
                            start=(kc == 0), stop=(kc == KC - 1))
                    if scale is not None:
                        nc.scalar.activation(ot[:, th * NH:(th + 1) * NH],
                                             ps[:, :], AF.Copy, scale=scale)
                    else:
                        nc.vector.tensor_copy(ot[:, th * NH:(th + 1) * NH],
                                              ps[:, :])
                outT.append(ot)
            return outT

        # out-projection + bias (rank-1 PSUM preload) + residual into xsb
        def proj_tok_residual(inT, w_tiles, bias_row, nm):
            with tc.tile_pool(name=f"psproj{nm}", bufs=3, space="PSUM") as psproj:
                for t in range(LT):
                    ps = psproj.tile([P, NH], F32, tag="ps", name="ps_proj")
                    nc.tensor.matmul(ps[:, :], lhsT=ones[0:1, 0:P],
                                     rhs=bias_row[:, :], start=True, stop=False)
                    for kc in range(KC):
                        nc.tensor.matmul(ps[:, :],
                                         lhsT=inT[kc][:, t * P:(t + 1) * P],
                                         rhs=w_tiles[kc][:, :],
                                         start=False, stop=(kc == KC - 1))
                    nc.vector.tensor_add(xsb[t][:, :], ps[:, :], xsb[t][:, :])

        # ==================================================================
        # Section 1: self-attention
        # ==================================================================
        h1T = layer_norm_T(0)

        es_s1 = ExitStack()
        vpool = es_s1.enter_context(tc.tile_pool(name="vsb", bufs=1))
        vsb = []
        with tc.tile_pool(name="psqkv", bufs=4, space="PSUM") as psqkv:
            qT = proj_featT(wq1, h1T, psqkv, "qT", scale=DH ** -0.5)
            kT = proj_featT(wk1, h1T, psqkv, "kT")
            for t in range(LT):
                vt = vpool.tile([P, H, DH + 1], BF16, tag=f"v{t}", name=f"v{t}")
                nc.vector.memset(vt[:, :, DH:DH + 1], 1.0)
                ps = psqkv.tile([P, NH], F32, tag="psv", name="ps_v")
                for kc in range(KC):
                    nc.tensor.matmul(ps[:, :],
                                     lhsT=h1T[kc][:, t * P:(t + 1) * P],
                                     rhs=wv1[kc][:, :],
                                     start=(kc == 0), stop=(kc == KC - 1))
                nc.vector.tensor_copy(vt[:, :, 0:DH],
                                      ps.rearrange("p (h d) -> p h d", h=H))
                vsb.append(vt)

        # cross-attn weights: DMA/cast now so they overlap attn1 compute
        es_a2 = ExitStack()
        a2pool = es_a2.enter_context(tc.tile_pool(name="a2w", bufs=1,
                                                  side="right"))
        a2stage = es_a2.enter_context(tc.tile_pool(name="a2stage", bufs=2,
                                                   side="right"))
        wq2 = load_w(a2pool, "a2_wq", D, dst_dtype=F32R)
        wo2 = load_w(a2pool, "a2_wo", D, dst_dtype=F32R)
        wk2 = load_w(a2pool, "a2_wk", CD, dst_dtype=BF16, stage=a2stage)
        wv2 = load_w(a2pool, "a2_wv", CD, dst_dtype=BF16, stage=a2stage)

        # attention: scores transposed (S^T = K Q^T), 64x128 row-tiled PE
        # (two heads run in the two SBUF partition halves); the softmax
        # denominator rides as V's extra ones-column; normalization via a
        # rank-1 PE broadcast of the reciprocal sums.
        o1T = [opool.tile([P, L], F32R, tag=f"oT{hp}", name=f"oT{hp}")
               for hp in range(KC)]
        with tc.tile_pool(name="expS", bufs=24) as espool, \
             tc.tile_pool(name="attnsb", bufs=2) as attnsb, \
             tc.tile_pool(name="pss", bufs=3, space="PSUM") as pss, \
             tc.tile_pool(name="psav", bufs=2, space="PSUM") as psav, \
             tc.tile_pool(name="psb", bufs=2, space="PSUM") as psb:
            for hp in range(KC):
                for th in range(TH):
                    es = {}
                    for lk in range(LT):
                        for sub in (0, 1):
                            ps_s = pss.tile([P, NH], F32, tag="ps", name="ps_s")
                            nc.tensor.matmul(
                                ps_s[:, :],
                                lhsT=kT[hp][sub * DH:(sub + 1) * DH,
                                            lk * P:(lk + 1) * P],
                                rhs=qT[hp][sub * DH:(sub + 1) * DH,
                                           th * NH:(th + 1) * NH],
                                start=True, stop=True)
                            e = espool.tile([P, NH], BF16, tag="e", name="expS")
                            nc.scalar.activation(e[:, :], ps_s[:, :], AF.Exp)
                            es[(sub, lk)] = e
                    for sub in (0, 1):
                        head = 2 * hp + sub
                        ps_o = psav.tile([P, NH], F32, tag="ps", name="ps_av")
                        for lk in range(LT):
                            nc.tensor.matmul(ps_o[0:DH + 1, :],
                                             lhsT=vsb[lk][:, head, :],
                                             rhs=es[(sub, lk)][:, :],
                                             start=(lk == 0), stop=(lk == LT - 1))
                        rec = attnsb.tile([1, NH], F32R, tag="rec", name="rec")
                        nc.vector.reciprocal(rec[:, :], ps_o[DH:DH + 1, :])
                        ps_b = psb.tile([P, NH], F32, tag="ps", name="ps_b")
                        nc.tensor.matmul(ps_b[0:DH, :], lhsT=ones[0:1, 0:DH],
                                         rhs=rec[:, :], start=True, stop=True)
                        rb = attnsb.tile([P, NH], F32, tag="rb", name="rb")
                        nc.scalar.activation(rb[0:DH, :], ps_b[0:DH, :], AF.Copy)
                        nc.vector.tensor_mul(
                            o1T[hp][sub * DH:(sub + 1) * DH,
                                    th * NH:(th + 1) * NH],
                            ps_o[0:DH, :], rb[0:DH, :])
        es_s1.close()

        proj_tok_residual(o1T, wo1, bo1_row, "1")
        es_a1.close()

        # ==================================================================
        # Section 2: cross-attention (keys/values from context, Lk = 77)
        # ==================================================================
        h2T = layer_norm_T(1)

        # FF2 weights: DMA/cast now (into space freed by a1w) to overlap attn2
        es_ffw = ExitStack()
        ffwpool = es_ffw.enter_context(tc.tile_pool(name="ffw", bufs=1))
        ffstage = es_ffw.enter_context(tc.tile_pool(name="ffstage", bufs=2))
        w2bf = []
        for j in range(FC):
            wf = ffstage.tile([P, D], F32, tag="stage", name=f"stg_w2_{j}")
            nc.sync.dma_start(out=wf[:, :], in_=pr["ff_w2"][j * P:(j + 1) * P, :])
            wb = ffwpool.tile([P, D], BF16, tag=f"w2bf{j}", name=f"w2bf{j}")
            nc.gpsimd.tensor_copy(wb[:, :], wf[:, :])
            w2bf.append(wb)

        es_s2 = ExitStack()
        s2pool = es_s2.enter_context(tc.tile_pool(name="s2", bufs=1))
        ctx = s2pool.tile([P, CD], F32, tag="ctx", name="ctx")
        nc.sync.dma_start(out=ctx[0:S, :], in_=pr["context"][:, :])

        with tc.tile_pool(name="psq2", bufs=2, space="PSUM") as psq2:
            q2T = proj_featT(wq2, h2T, psq2, "qT", scale=DH ** -0.5)

            ctxT = []
            for cc in range(CC):
                ct = qkpool.tile([P, S], BF16, tag=f"ctxT{cc}", name=f"ctxT{cc}")
                ps = psq2.tile([P, P], F32, tag="pst", name="ps_ctxT")
                nc.tensor.transpose(ps[:, 0:S], ctx[0:S, cc * P:(cc + 1) * P],
                                    identity[0:S, 0:S])
                nc.vector.tensor_copy(ct[:, :], ps[:, 0:S])
                ctxT.append(ct)

            k2T = []
            for oc in range(KC):
                kt = qkpool.tile([P, S], BF16, tag=f"k2T{oc}", name=f"k2T{oc}")
                ps = psq2.tile([P, P], F32, tag="pst", name="ps_k2T")
                for cc in range(CC):
                    nc.tensor.matmul(ps[:, 0:S],
                                     lhsT=wk2[cc][:, oc * P:(oc + 1) * P],
                                     rhs=ctxT[cc][:, :],
                                     start=(cc == 0), stop=(cc == CC - 1))
                nc.vector.tensor_copy(kt[:, :], ps[:, 0:S])
                k2T.append(kt)

            v2 = s2pool.tile([P, H, DH + 1], BF16, tag="v2", name="v2")
            nc.vector.memset(v2[0:S, :, DH:DH + 1], 1.0)
            ps = psq2.tile([P, NH], F32, tag="psv2", name="ps_v2")
            for cc in range(CC):
                nc.tensor.matmul(ps[0:S, :], lhsT=ctxT[cc][:, :],
                                 rhs=wv2[cc][:, :],
                                 start=(cc == 0), stop=(cc == CC - 1))
            nc.vector.tensor_copy(v2[0:S, :, 0:DH],
                                  ps[0:S, :].rearrange("p (h d) -> p h d", h=H))

        o2T = [opool.tile([P, L], F32R, tag=f"oT{hp}", name=f"o2T{hp}")
               for hp in range(KC)]
        with tc.tile_pool(name="expS2", bufs=6) as es2pool, \
             tc.tile_pool(name="attnsb2", bufs=2) as attnsb2, \
             tc.tile_pool(name="pss2", bufs=3, space="PSUM") as pss2, \
             tc.tile_pool(name="psav2", bufs=2, space="PSUM") as psav2, \
             tc.tile_pool(name="psb2", bufs=2, space="PSUM") as psb2:
            for hp in range(KC):
                for th in range(TH):
                    for sub in (0, 1):
                        head = 2 * hp + sub
                        ps_s = pss2.tile([P, NH], F32, tag="ps", name="ps_s2")
                        nc.tensor.matmul(
                            ps_s[0:S, :],
                            lhsT=k2T[hp][sub * DH:(sub + 1) * DH, :],
                            rhs=q2T[hp][sub * DH:(sub + 1) * DH,
                                        th * NH:(th + 1) * NH],
                            start=True, stop=True)
                        e = es2pool.tile([P, NH], BF16, tag="e", name="expS2")
                        nc.scalar.activation(e[0:S, :], ps_s[0:S, :], AF.Exp)
                        ps_o = psav2.tile([P, NH], F32, tag="ps", name="ps_av2")
                        nc.tensor.matmul(ps_o[0:DH + 1, :],
                                         lhsT=v2[0:S, head, :],
                                         rhs=e[0:S, :], start=True, stop=True)
                        rec = attnsb2.tile([1, NH], F32R, tag="rec", name="rec2")
                        nc.vector.reciprocal(rec[:, :], ps_o[DH:DH + 1, :])
                        ps_b = psb2.tile([P, NH], F32, tag="ps", name="ps_b2")
                        nc.tensor.matmul(ps_b[0:DH, :], lhsT=ones[0:1, 0:DH],
                                         rhs=rec[:, :], start=True, stop=True)
                        rb = attnsb2.tile([P, NH], F32, tag="rb", name="rb2")
                        nc.scalar.activation(rb[0:DH, :], ps_b[0:DH, :], AF.Copy)
                        nc.vector.tensor_mul(
                            o2T[hp][sub * DH:(sub + 1) * DH,
                                    th * NH:(th + 1) * NH],
                            ps_o[0:DH, :], rb[0:DH, :])
        es_s2.close()

        proj_tok_residual(o2T, wo2, bo2_row, "2")
        es_a2.close()

        # ==================================================================
        # Section 3: GEGLU feed-forward
        # ==================================================================
        h3T = layer_norm_T(2)

        es_s3 = ExitStack()
        prodpool = es_s3.enter_context(tc.tile_pool(name="prod", bufs=1))
        fftmp = es_s3.enter_context(tc.tile_pool(name="fftmp", bufs=2))
        ffpiece = es_s3.enter_context(tc.tile_pool(name="ffpiece", bufs=2))

        # ff_w1 is read exactly once by PE: stream it as [128,512] pieces
        # (4 column-chunks per DMA -- full 2KB partition lines, deeper prefetch)
        def ff1_pieces(group, base):
            pieces = []
            for kc in range(KC):
                pc = ffpiece.tile([P, NH], F32R, tag=f"fp{kc}",
                                  name=f"ffw1_{base}_{group}_{kc}")
                nc.sync.dma_start(
                    out=pc[:, :],
                    in_=pr["ff_w1"][kc * P:(kc + 1) * P,
                                    base + group * NH:base + (group + 1) * NH])
                pieces.append(pc)
            return pieces

        prod = [prodpool.tile([P, L], BF16, tag=f"prod{j}", name=f"prod{j}")
                for j in range(FC)]
        with tc.tile_pool(name="psff", bufs=4, space="PSUM") as psff:
            for g in range(FC // 4):
              vpieces = ff1_pieces(g, 0)
              gpieces = ff1_pieces(g, 2 * FF // 2)
              for jj in range(4):
                j = g * 4 + jj
                val = fftmp.tile([P, L], F32, tag="val", name=f"val{j}")
                gel = fftmp.tile([P, L], F32, tag="gel", name=f"gel{j}")
                for th in range(TH):
                    ps_v = psff.tile([P, NH], F32, tag="ps", name="ps_ffv")
                    for kc in range(KC):
                        nc.tensor.matmul(
                            ps_v[:, :],
                            lhsT=vpieces[kc][:, jj * P:(jj + 1) * P],
                            rhs=h3T[kc][:, th * NH:(th + 1) * NH],
                            start=(kc == 0), stop=(kc == KC - 1))
                    nc.vector.tensor_scalar(val[:, th * NH:(th + 1) * NH],
                                            ps_v[:, :], fb1[:, j:j + 1], None,
                                            op0=ALU.add)
                    ps_g = psff.tile([P, NH], F32, tag="ps", name="ps_ffg")
                    for kc in range(KC):
                        nc.tensor.matmul(
                            ps_g[:, :],
                            lhsT=gpieces[kc][:, jj * P:(jj + 1) * P],
                            rhs=h3T[kc][:, th * NH:(th + 1) * NH],
                            start=(kc == 0), stop=(kc == KC - 1))
                    nc.scalar.activation(gel[:, th * NH:(th + 1) * NH], ps_g[:, :],
                                         AF.Gelu, bias=fb1[:, FC + j:FC + j + 1])
                nc.vector.tensor_mul(prod[j][:, :], val[:, :], gel[:, :])

        with tc.tile_pool(name="psff2", bufs=3, space="PSUM") as psff2:
            for t in range(LT):
                ps = psff2.tile([P, NH], F32, tag="ps", name="ps_ff2")
                nc.tensor.matmul(ps[:, :], lhsT=ones[0:1, 0:P],
                                 rhs=fb2_row[:, :], start=True, stop=False)
                for j in range(FC):
                    nc.tensor.matmul(ps[:, :],
                                     lhsT=prod[j][:, t * P:(t + 1) * P],
                                     rhs=w2bf[j][:, :],
                                     start=False, stop=(j == FC - 1))
                nc.vector.tensor_add(xsb[t][:, :], ps[:, :], xsb[t][:, :])
                nc.sync.dma_start(out=out_p[t * P:(t + 1) * P, :],
                                  in_=xsb[t][:, :])
        es_s3.close()
        es_ffw.close()


_NC_CACHE = {}


def _get_nc():
    if "nc" not in _NC_CACHE:
        _NC_CACHE["nc"] = _build_nc()
    return _NC_CACHE["nc"]


def kernel(**inputs):
    nc = _get_nc()
    x = np.asarray(inputs["x"], dtype=np.float32)
    ctx = np.asarray(inputs["context"], dtype=np.float32)
    shared = {k: np.asarray(v, dtype=np.float32) for k, v in inputs.items()
              if k not in ("x", "context")}
    in_maps = []
    for b in range(NCORES):
        m = {"x": np.ascontiguousarray(x[b]),
             "context": np.ascontiguousarray(ctx[b])}
        m.update(shared)
        in_maps.append(m)
    res = run_bass_kernel_spmd(nc, in_maps, list(range(NCORES)))
    out = np.stack([res.results[i]["out"] for i in range(NCORES)], axis=0)
    return out.astype(np.float32)



# revision 2
# speedup vs baseline: 217.1078x; 217.1078x over previous
"""v2 Trainium2 Bass kernel for BasicTransformerBlock — restructured from the
baseline after HW section profiling:

- attention: wide 2-bank score tiles + one [*,1024] exp per (sub,lk); AV into
  4 parallel PSUM accumulators; softmax normalization via DVE recip + DMA
  partition-broadcast + one DVE mul (ACT does exps only).
- FF: val bias via rank-1 PSUM preload; gelu + product as ONE wide ACT op and
  ONE DVE mul reading val straight out of PSUM (no val staging tile).
- bf16 weights/activations everywhere off the LN-stats path (casts on GpSimd).
- 1/sqrt(dh) folded into the exp scale.

Data-parallel over batch B=8 across 8 NeuronCores.
"""

import os

import numpy as np

import concourse.bass as bass
import concourse.tile as tile
from concourse import library_config, mybir
from concourse.bass_utils import run_bass_kernel_spmd
from concourse.masks import make_identity

P = 128
B, L, D, S, CD, H, DH = 8, 1024, 512, 77, 768, 8, 64
FF = 2048
LT = L // P               # 8 token tiles
KC = D // P               # 4 model-dim chunks
CC = CD // P              # 6 context-dim chunks
NH = 512                  # PSUM bank width (fp32)
TH = L // NH              # 2 token halves
FC = FF // P              # 16 ff chunks
EPS = 1e-5
NCORES = 8

F32 = mybir.dt.float32
F32R = mybir.dt.float32r
BF16 = mybir.dt.bfloat16
AF = mybir.ActivationFunctionType
ALU = mybir.AluOpType

SCALE = DH ** -0.5


def _legalize_wait_counts(nc, max_waits=1):
    n_moved = 0
    for f in nc.m.functions:
        for bb in f.blocks:
            out, changed = [], False
            for inst in bb.instructions:
                si = inst.sync_info
                waits = list(si.on_wait) if si is not None and si.on_wait else []
                cap = 2 if isinstance(inst, mybir.InstEventSemaphore) else max_waits
                if len(waits) > cap:
                    keep, rest = waits[:cap], waits[cap:]
                    for i in range(0, len(rest), max_waits):
                        out.append(mybir.InstNoOp(
                            name=f"{inst.name}-lw{i}",
                            engine=inst.engine,
                            bass_nofuse=True,
                            sync_info=mybir.SyncInfo(
                                on_wait=rest[i:i + max_waits], on_update=[]),
                        ))
                    si.on_wait = keep
                    inst.sync_info = si
                    n_moved += len(rest)
                    changed = True
                out.append(inst)
            if changed:
                bb.instructions = out
    return n_moved


def _build_nc():
    nc = bass.Bass(target_bir_lowering=False, debug=True)

    pr = {}
    pr["x"] = nc.declare_dram_parameter("x", [L, D], F32, isOutput=False)
    pr["context"] = nc.declare_dram_parameter("context", [S, CD], F32, isOutput=False)
    for nm in ("ln1_g", "ln1_b", "ln2_g", "ln2_b", "ln3_g", "ln3_b"):
        pr[nm] = nc.declare_dram_parameter(nm, [D], F32, isOutput=False)
    for nm in ("a1_bo", "a2_bo", "ff_b2"):
        pr[nm] = nc.declare_dram_parameter(nm, [D], F32R, isOutput=False)
    for nm in ("a1_wq", "a1_wk", "a1_wv", "a2_wq"):
        pr[nm] = nc.declare_dram_parameter(nm, [D, D], F32R, isOutput=False)
    for nm in ("a1_wo", "a2_wo"):
        pr[nm] = nc.declare_dram_parameter(nm, [D, D], F32, isOutput=False)
    for nm in ("a2_wk", "a2_wv"):
        pr[nm] = nc.declare_dram_parameter(nm, [CD, D], F32, isOutput=False)
    pr["ff_w1"] = nc.declare_dram_parameter("ff_w1", [D, 2 * FF], F32R, isOutput=False)
    pr["ff_b1"] = nc.declare_dram_parameter("ff_b1", [2 * FF], F32, isOutput=False)
    pr["ff_w2"] = nc.declare_dram_parameter("ff_w2", [FF, D], F32, isOutput=False)
    out_p = nc.declare_dram_parameter("out", [L, D], F32, isOutput=True)

    reps = int(os.environ.get("BASS_KERNEL_REPS", "1"))
    with tile.TileContext(nc) as tc, \
         nc.allow_low_precision(reason="bf16 matmul operands with fp32 PSUM "
                                       "accumulation; tolerance is 2e-2"):
        if reps > 1:
            with tc.For_i(0, reps, 1):
                _emit(nc, tc, pr, out_p)
        else:
            _emit(nc, tc, pr, out_p)
    if os.environ.get("BASS_KERNEL_SKIP_WAIT_LEGALIZE") != "1":
        _legalize_wait_counts(nc)
    return nc


def _emit(nc, tc, pr, out_p):
    from contextlib import ExitStack

    CUT = int(os.environ.get("BASS_CUT", "3"))
    S1 = int(os.environ.get("BASS_S1", "7"))
    EXPW = int(os.environ.get("BASS_EXPW", str(L)))

    top = ExitStack()
    with top:
        # ------------------ persistent pools -------------------------------
        const = top.enter_context(tc.tile_pool(name="const", bufs=1))
        xpool = top.enter_context(tc.tile_pool(name="xsb", bufs=1))
        htpool = top.enter_context(tc.tile_pool(name="hT", bufs=1))
        lntp = top.enter_context(tc.tile_pool(name="lnt", bufs=3))
        qkpool = top.enter_context(tc.tile_pool(name="qkT", bufs=1))
        opool = top.enter_context(tc.tile_pool(name="oT", bufs=1))
        stage = top.enter_context(tc.tile_pool(name="stage", bufs=3))

        # ----------------------------- constants ---------------------------
        identity = const.tile([P, P], F32, tag="identity", name="identity")
        make_identity(nc, identity[:, :])
        ident_bf = const.tile([P, P], BF16, tag="ident_bf", name="ident_bf")
        nc.gpsimd.tensor_copy(ident_bf[:, :], identity[:, :])
        ones_f = const.tile([1, NH], F32, tag="ones_f", name="ones_f")
        nc.vector.memset(ones_f[:, :], 1.0)
        ones = const.tile([1, NH], F32R, tag="ones", name="ones")
        nc.vector.tensor_copy(ones[:, :], ones_f[:, :])
        eps_t = const.tile([P, 1], F32, tag="eps", name="eps")
        nc.vector.memset(eps_t[:, :], EPS)

        # ------------------------------ x in -------------------------------
        xsb = []
        for t in range(LT):
            xt = xpool.tile([P, D], F32, tag=f"x{t}", name=f"x{t}")
            nc.sync.dma_start(out=xt[:, :], in_=pr["x"][t * P:(t + 1) * P, :])
            xsb.append(xt)

        def flush_out():
            for t in range(LT):
                nc.sync.dma_start(out=out_p[t * P:(t + 1) * P, :],
                                  in_=xsb[t][:, :])

        def load_w_bf(pool, nm, rows, tag=None):
            """DMA f32 rows, cast to bf16 on GpSimd via the shared stage."""
            tiles = []
            for c in range(rows // P):
                wf = stage.tile([P, D], F32, tag="stage", name=f"stg_{nm}{c}")
                nc.sync.dma_start(out=wf[:, :], in_=pr[nm][c * P:(c + 1) * P, :])
                w = pool.tile([P, D], BF16, tag=f"{tag or nm}{c}",
                              name=f"{nm}{c}")
                nc.gpsimd.tensor_copy(w[:, :], wf[:, :])
                tiles.append(w)
            return tiles

        def load_w_direct(pool, nm, rows, tag=None):
            """DMA F32R rows straight into SBUF (no cast latency)."""
            tiles = []
            for c in range(rows // P):
                w = pool.tile([P, D], F32R, tag=f"{tag or nm}{c}",
                              name=f"{nm}{c}")
                nc.sync.dma_start(out=w[:, :], in_=pr[nm][c * P:(c + 1) * P, :])
                tiles.append(w)
            return tiles

        es_a1 = ExitStack()
        a1pool = es_a1.enter_context(tc.tile_pool(name="a1w", bufs=1))
        es_a1qkv = ExitStack()
        a1qkv = es_a1qkv.enter_context(tc.tile_pool(name="a1qkv", bufs=1,
                                                    side="right"))
        wq1 = load_w_direct(a1qkv, "a1_wq", D)
        wk1 = load_w_direct(a1qkv, "a1_wk", D)
        wv1 = load_w_direct(a1qkv, "a1_wv", D)
        wo1 = load_w_bf(a1pool, "a1_wo", D)

        # LN gamma/beta stacked -> gb [128, 24]; ff_b1 -> fb1 [128, 32]
        lnstack = const.tile([P, P], F32, tag="lnstack", name="lnstack")
        for i, (gnm, bnm) in enumerate((("ln1_g", "ln1_b"), ("ln2_g", "ln2_b"),
                                        ("ln3_g", "ln3_b"))):
            nc.sync.dma_start(out=lnstack[i * 8:i * 8 + 4, :],
                              in_=pr[gnm].rearrange("(a f) -> a f", f=P))
            nc.sync.dma_start(out=lnstack[i * 8 + 4:i * 8 + 8, :],
                              in_=pr[bnm].rearrange("(a f) -> a f", f=P))
        fb1stack = const.tile([32, P], F32, tag="fb1stack", name="fb1stack")
        nc.sync.dma_start(out=fb1stack[:, :],
                          in_=pr["ff_b1"].rearrange("(a f) -> a f", f=P))
        fb1row = const.tile([1, 2 * FF], F32, tag="fb1row", name="fb1row")
        nc.sync.dma_start(out=fb1row[:, :],
                          in_=pr["ff_b1"].rearrange("(o f) -> o f", o=1))
        fb1r = const.tile([1, 2 * FF], F32R, tag="fb1r", name="fb1r")
        nc.vector.tensor_copy(fb1r[:, :], fb1row[:, :])
        bo1_row = const.tile([1, D], F32R, tag="bo1row", name="bo1row")
        nc.sync.dma_start(out=bo1_row[:, :],
                          in_=pr["a1_bo"].rearrange("(o f) -> o f", o=1))
        bo2_row = const.tile([1, D], F32R, tag="bo2row", name="bo2row")
        nc.sync.dma_start(out=bo2_row[:, :],
                          in_=pr["a2_bo"].rearrange("(o f) -> o f", o=1))
        fb2_row = const.tile([1, D], F32R, tag="fb2row", name="fb2row")
        nc.sync.dma_start(out=fb2_row[:, :],
                          in_=pr["ff_b2"].rearrange("(o f) -> o f", o=1))

        gb = const.tile([P, 24], F32, tag="gb", name="gb")
        fb1 = const.tile([P, 32], F32, tag="fb1", name="fb1")
        with tc.tile_pool(name="psprep", bufs=2, space="PSUM") as psprep:
            pst = psprep.tile([P, P], F32, tag="pstr", name="pstr_gb")
            nc.tensor.transpose(pst[:, 0:24], lnstack[0:24, :], identity[0:24, 0:24])
            nc.vector.tensor_copy(gb[:, :], pst[:, 0:24])
            pst2 = psprep.tile([P, P], F32, tag="pstr", name="pstr_fb1")
            nc.tensor.transpose(pst2[:, 0:32], fb1stack[0:32, :], identity[0:32, 0:32])
            nc.vector.tensor_copy(fb1[:, :], pst2[:, 0:32])

        def g_col(i, kc):
            return gb[:, i * 8 + kc:i * 8 + kc + 1]

        def b_col(i, kc):
            return gb[:, i * 8 + 4 + kc:i * 8 + 4 + kc + 1]

        # ------------------------------------------------------------------
        # LayerNorm -> feature-major bf16 hT [128ch, 1024tok] x 4
        # ------------------------------------------------------------------
        def layer_norm_T(ln_i):
            hT = [htpool.tile([P, L], F32R, tag=f"hT{c}", name=f"hT{ln_i}_{c}")
                  for c in range(KC)]
            with tc.tile_pool(name=f"lnps{ln_i}", bufs=3, space="PSUM") as lnps:
                for t in range(LT):
                    st = lntp.tile([P, 6], F32, tag="bnstats", name="bnstats")
                    nc.vector.bn_stats(st[:, :], xsb[t][:, :])
                    mv = lntp.tile([P, 2], F32, tag="bnaggr", name="bnaggr")
                    nc.vector.bn_aggr(mv[:, :], st[:, :])
                    lnv = lntp.tile([P, 1], F32, tag="lnv", name="lnv")
                    nc.scalar.activation(lnv[:, :], mv[:, 1:2], AF.Ln,
                                         bias=eps_t[:, :])
                    rstd = lntp.tile([P, 1], F32, tag="rstd", name="rstd")
                    nc.scalar.activation(rstd[:, :], lnv[:, :], AF.Exp, scale=-0.5)
                    nmr = lntp.tile([P, 1], F32, tag="nmr", name="nmr")
                    nc.vector.tensor_scalar(nmr[:, :], mv[:, 0:1], rstd[:, :], -1.0,
                                            op0=ALU.mult, op1=ALU.mult)
                    z = lntp.tile([P, D], BF16, tag="z", name="z")
                    nc.scalar.activation(z[:, :], xsb[t][:, :], AF.Identity,
                                         bias=nmr[:, :], scale=rstd[:, :])
                    for c in range(KC):
                        ps = lnps.tile([P, P], BF16, tag="ps", name="lntr")
                        nc.tensor.transpose(ps[:, :], z[:, c * P:(c + 1) * P],
                                            ident_bf[:, :])
                        nc.vector.tensor_scalar(hT[c][:, t * P:(t + 1) * P],
                                                ps[:, :],
                                                g_col(ln_i, c), b_col(ln_i, c),
                                                op0=ALU.mult, op1=ALU.add)
            return hT

        # weight-stationary projection: out feature-major bf16 [128, 1024]x4
        # 8 PSUM banks as (oc, th) accumulators; 1 LDW per (kc, oc).
        def proj_featT(w_tiles, hT, pspool, out_tag, name_pfx=None):
            name_pfx = name_pfx or out_tag
            outT = [qkpool.tile([P, L], BF16, tag=f"{out_tag}{oc}",
                                name=f"{name_pfx}{oc}") for oc in range(KC)]
            psb = [pspool.tile([P, L], F32, tag=f"ps{oc}", name=f"ps_{out_tag}{oc}")
                   for oc in range(KC)]
            for kc in range(KC):
                for oc in range(KC):
                    for th in range(TH):
                        nc.tensor.matmul(
                            psb[oc][:, th * NH:(th + 1) * NH],
                            lhsT=w_tiles[kc][:, oc * P:(oc + 1) * P],
                            rhs=hT[kc][:, th * NH:(th + 1) * NH],
                            start=(kc == 0), stop=(kc == KC - 1))
            for oc in range(KC):
                nc.vector.tensor_copy(outT[oc][:, :], psb[oc][:, :])
            return outT

        # token-major out-projection + bias + residual into xsb
        def proj_tok_residual(inT, w_tiles, bias_row, nm):
            with tc.tile_pool(name=f"psproj{nm}", bufs=3, space="PSUM") as psproj:
                for t in range(LT):
                    ps = psproj.tile([P, NH], F32, tag="ps", name="ps_proj")
                    nc.tensor.matmul(ps[:, :], lhsT=ones[0:1, 0:P],
                                     rhs=bias_row[:, :], start=True, stop=False)
                    for kc in range(KC):
                        nc.tensor.matmul(ps[:, :],
                                         lhsT=inT[kc][:, t * P:(t + 1) * P],
                                         rhs=w_tiles[kc][:, :],
                                         start=False, stop=(kc == KC - 1))
                    nc.vector.tensor_add(xsb[t][:, :], ps[:, :], xsb[t][:, :])

        # attention core (shared by attn1/attn2): transposed scores + ones-row
        # denominator; normalization = DVE recip -> PE rank-1 broadcast ->
        # DVE evict -> DVE mul (ACT stays exp-only)
        def attn_core(oT, kTt, qTt, vget, nkeys, kslice, espool, attnsb,
                      rbpool, pss, psav, psb, nm):
            SK = nkeys
            for hp in range(KC):
                es = {}
                for lk in range(kslice):
                    for sub in (0, 1):
                        ps_s = pss.tile([P, L], F32, tag="ps", name=f"ps_s{nm}")
                        for th in range(TH):
                            nc.tensor.matmul(
                                ps_s[0:SK, th * NH:(th + 1) * NH],
                                lhsT=kTt[hp][sub * DH:(sub + 1) * DH,
                                             lk * P:lk * P + SK],
                                rhs=qTt[hp][sub * DH:(sub + 1) * DH,
                                            th * NH:(th + 1) * NH],
                                start=True, stop=True)
                        e = espool.tile([P, L], BF16, tag="e", name=f"expS{nm}")
                        if EXPW >= L:
                            nc.scalar.activation(e[0:SK, :], ps_s[0:SK, :],
                                                 AF.Exp, scale=SCALE)
                        else:
                            for th in range(TH):
                                nc.scalar.activation(
                                    e[0:SK, th * NH:(th + 1) * NH],
                                    ps_s[0:SK, th * NH:(th + 1) * NH],
                                    AF.Exp, scale=SCALE)
                        es[(sub, lk)] = e
                for sub in (0, 1):
                    head = 2 * hp + sub
                    ps_o = [psav.tile([P, NH], F32, tag="ps", name=f"ps_av{nm}")
                            for th in range(TH)]
                    for lk in range(kslice):
                        vt = vget(lk, head)
                        for th in range(TH):
                            nc.tensor.matmul(
                                ps_o[th][0:DH + 1, :],
                                lhsT=vt,
                                rhs=es[(sub, lk)][0:SK, th * NH:(th + 1) * NH],
                                start=(lk == 0), stop=(lk == kslice - 1))
                    for th in range(TH):
                        rec = attnsb.tile([1, NH], F32R, tag="rec", name=f"rec{nm}")
                        nc.vector.reciprocal(rec[:, :], ps_o[th][DH:DH + 1, :])
                        ps_b = psb.tile([DH, NH], F32, tag="psb", name=f"ps_b{nm}")
                        nc.tensor.matmul(ps_b[:, :], lhsT=ones[0:1, 0:DH],
                                         rhs=rec[:, :], start=True, stop=True)
                        rb = rbpool.tile([DH, NH], F32, tag="rb", name=f"rb{nm}")
                        nc.vector.tensor_copy(rb[:, :], ps_b[:, :])
                        nc.vector.tensor_mul(
                            oT[hp][sub * DH:(sub + 1) * DH,
                                   th * NH:(th + 1) * NH],
                            ps_o[th][0:DH, :], rb[:, :])

        # ==================================================================
        # Section 1: self-attention
        # ==================================================================
        h1T = layer_norm_T(0)

        es_s1 = ExitStack()
        vpool = es_s1.enter_context(tc.tile_pool(name="vsb", bufs=1))
        with tc.tile_pool(name="psqk", bufs=1, space="PSUM") as psqk:
            qT = proj_featT(wq1, h1T, psqk, "qT")
            kT = proj_featT(wk1, h1T, psqk, "kT")
        vsb = []
        with tc.tile_pool(name="psv", bufs=4, space="PSUM") as psv:
            for t in range(LT):
                vt = vpool.tile([P, H, DH + 1], BF16, tag=f"v{t}", name=f"v{t}")
                nc.vector.memset(vt[:, :, DH:DH + 1], 1.0)
                ps = psv.tile([P, NH], F32, tag="psv", name="ps_v")
                for kc in range(KC):
                    nc.tensor.matmul(ps[:, :],
                                     lhsT=h1T[kc][:, t * P:(t + 1) * P],
                                     rhs=wv1[kc][:, :],
                                     start=(kc == 0), stop=(kc == KC - 1))
                nc.vector.tensor_copy(vt[:, :, 0:DH],
                                      ps.rearrange("p (h d) -> p h d", h=H))
                vsb.append(vt)
        es_a1qkv.close()

        # cross-attn weights + context prep (done pre-attn1; small PE cost)
        es_a2 = ExitStack()
        a2pool = es_a2.enter_context(tc.tile_pool(name="a2w", bufs=1,
                                                  side="right"))
        wq2 = load_w_direct(a2pool, "a2_wq", D)
        wo2 = load_w_bf(a2pool, "a2_wo", D)
        wk2 = load_w_bf(a2pool, "a2_wk", CD)
        wv2 = load_w_bf(a2pool, "a2_wv", CD)

        s2pool = es_a2.enter_context(tc.tile_pool(name="s2", bufs=1,
                                                  side="right"))
        ctx = s2pool.tile([P, CD], F32, tag="ctx", name="ctx")
        nc.sync.dma_start(out=ctx[0:S, :], in_=pr["context"][:, :])

        with tc.tile_pool(name="psq2", bufs=2, space="PSUM") as psq2:
            ctxT = []
            for cc in range(CC):
                ct = s2pool.tile([P, S], BF16, tag=f"ctxT{cc}", name=f"ctxT{cc}")
                ps = psq2.tile([P, P], F32, tag="pst", name="ps_ctxT")
                nc.tensor.transpose(ps[:, 0:S], ctx[0:S, cc * P:(cc + 1) * P],
                                    identity[0:S, 0:S])
                nc.vector.tensor_copy(ct[:, :], ps[:, 0:S])
                ctxT.append(ct)
            k2T = []
            for oc in range(KC):
                kt = s2pool.tile([P, S], BF16, tag=f"k2T{oc}", name=f"k2T{oc}")
                ps = psq2.tile([P, P], F32, tag="pst", name="ps_k2T")
                for cc in range(CC):
                    nc.tensor.matmul(ps[:, 0:S],
                                     lhsT=wk2[cc][:, oc * P:(oc + 1) * P],
                                     rhs=ctxT[cc][:, :],
                                     start=(cc == 0), stop=(cc == CC - 1))
                nc.vector.tensor_copy(kt[:, :], ps[:, 0:S])
                k2T.append(kt)
            v2 = s2pool.tile([P, H, DH + 1], BF16, tag="v2", name="v2")
            nc.vector.memset(v2[0:S, :, DH:DH + 1], 1.0)
            ps = psq2.tile([P, NH], F32, tag="psv2", name="ps_v2")
            for cc in range(CC):
                nc.tensor.matmul(ps[0:S, :], lhsT=ctxT[cc][:, :],
                                 rhs=wv2[cc][:, :],
                                 start=(cc == 0), stop=(cc == CC - 1))
            nc.vector.tensor_copy(v2[0:S, :, 0:DH],
                                  ps[0:S, :].rearrange("p (h d) -> p h d", h=H))

        if S1 <= 3:
            es_a2.close()
            es_s1.close()
            es_a1.close()
            flush_out()
            return

        # attn1 proper
        o1T = [opool.tile([P, L], BF16, tag=f"oT{hp}", name=f"oT{hp}")
               for hp in range(KC)]
        with tc.tile_pool(name="expS", bufs=16) as espool, \
             tc.tile_pool(name="attnsb", bufs=3) as attnsb, \
             tc.tile_pool(name="rbp", bufs=2) as rbpool, \
             tc.tile_pool(name="pss", bufs=2, space="PSUM") as pss, \
             tc.tile_pool(name="psav", bufs=3, space="PSUM") as psav, \
             tc.tile_pool(name="psb", bufs=1, space="PSUM") as psb:
            attn_core(o1T, kT, qT,
                      lambda lk, head: vsb[lk][:, head, :],
                      P, LT, espool, attnsb, rbpool, pss, psav, psb, "1")
        es_s1.close()

        if S1 <= 6:
            es_a2.close()
            es_a1.close()
            flush_out()
            return

        proj_tok_residual(o1T, wo1, bo1_row, "1")
        es_a1.close()

        if CUT == 1:
            es_a2.close()
            flush_out()
            return

        # ==================================================================
        # Section 2: cross-attention
        # ==================================================================
        h2T = layer_norm_T(1)

        # FF weights: DMA/cast now to overlap attn2
        if CUT >= 3:
            es_ffw = ExitStack()
            ffwpool = es_ffw.enter_context(tc.tile_pool(name="ffw", bufs=1))
            w2bf = load_w_bf(ffwpool, "ff_w2", FF, tag="w2bf")

        with tc.tile_pool(name="psq2b", bufs=1, space="PSUM") as psq2b:
            q2T = proj_featT(wq2, h2T, psq2b, "qT", name_pfx="q2T")

        o2T = [opool.tile([P, L], BF16, tag=f"oT{hp}", name=f"o2T{hp}")
               for hp in range(KC)]
        with tc.tile_pool(name="expS2", bufs=6) as es2pool, \
             tc.tile_pool(name="attnsb2", bufs=4) as attnsb2, \
             tc.tile_pool(name="rbp2", bufs=4) as rbpool2, \
             tc.tile_pool(name="pss2", bufs=2, space="PSUM") as pss2, \
             tc.tile_pool(name="psav2", bufs=3, space="PSUM") as psav2, \
             tc.tile_pool(name="psb2", bufs=1, space="PSUM") as psb2:
            attn_core(o2T, k2T, q2T,
                      lambda lk, head: v2[0:S, head, :],
                      S, 1, es2pool, attnsb2, rbpool2, pss2, psav2, psb2, "2")

        proj_tok_residual(o2T, wo2, bo2_row, "2")
        es_a2.close()

        if CUT == 2:
            flush_out()
            return

        # ==================================================================
        # Section 3: GEGLU feed-forward
        # ==================================================================
        h3T = layer_norm_T(2)

        es_s3 = ExitStack()
        prodpool = es_s3.enter_context(tc.tile_pool(name="prod", bufs=1))
        fftmp = es_s3.enter_context(tc.tile_pool(name="fftmp", bufs=2))
        ffpiece = es_s3.enter_context(tc.tile_pool(name="ffpiece", bufs=2))

        def ff1_pieces(group, base):
            pieces = []
            for kc in range(KC):
                pc = ffpiece.tile([P, NH], F32R, tag=f"fp{kc}",
                                  name=f"ffw1_{base}_{group}_{kc}")
                nc.sync.dma_start(
                    out=pc[:, :],
                    in_=pr["ff_w1"][kc * P:(kc + 1) * P,
                                    base + group * NH:base + (group + 1) * NH])
                pieces.append(pc)
            return pieces

        prod = [prodpool.tile([P, L], BF16, tag=f"prod{j}", name=f"prod{j}")
                for j in range(FC)]
        with tc.tile_pool(name="psff", bufs=2, space="PSUM") as psff:
            for g in range(FC // 4):
                vpieces = ff1_pieces(g, 0)
                gpieces = ff1_pieces(g, FF)
                for jj in range(4):
                    j = g * 4 + jj
                    ps_v = psff.tile([P, L], F32, tag="psv", name=f"ps_ffv{j}")
                    ps_g = psff.tile([P, L], F32, tag="psg", name=f"ps_ffg{j}")
                    # val bias preload (both th halves, one stationary)
                    for th in range(TH):
                        nc.tensor.matmul(ps_v[:, th * NH:(th + 1) * NH],
                                         lhsT=fb1r[0:1, j * P:(j + 1) * P],
                                         rhs=ones[0:1, :],
                                         start=True, stop=False)
                    for kc in range(KC):
                        for th in range(TH):
                            nc.tensor.matmul(
                                ps_v[:, th * NH:(th + 1) * NH],
                                lhsT=vpieces[kc][:, jj * P:(jj + 1) * P],
                                rhs=h3T[kc][:, th * NH:(th + 1) * NH],
                                start=False, stop=(kc == KC - 1))
                    for kc in range(KC):
                        for th in range(TH):
                            nc.tensor.matmul(
                                ps_g[:, th * NH:(th + 1) * NH],
                                lhsT=gpieces[kc][:, jj * P:(jj + 1) * P],
                                rhs=h3T[kc][:, th * NH:(th + 1) * NH],
                                start=(kc == 0), stop=(kc == KC - 1))
                    gel = fftmp.tile([P, L], BF16, tag="gel", name=f"gel{j}")
                    nc.scalar.activation(gel[:, :], ps_g[:, :], AF.Gelu,
                                         bias=fb1[:, FC + j:FC + j + 1])
                    nc.vector.tensor_mul(prod[j][:, :], ps_v[:, :], gel[:, :])

        with tc.tile_pool(name="psff2", bufs=3, space="PSUM") as psff2:
            for t in range(LT):
                ps = psff2.tile([P, NH], F32, tag="ps", name="ps_ff2")
                nc.tensor.matmul(ps[:, :], lhsT=ones[0:1, 0:P],
                                 rhs=fb2_row[:, :], start=True, stop=False)
                for j in range(FC):
                    nc.tensor.matmul(ps[:, :],
                                     lhsT=prod[j][:, t * P:(t + 1) * P],
                                     rhs=w2bf[j][:, :],
                                     start=False, stop=(j == FC - 1))
                nc.vector.tensor_add(xsb[t][:, :], ps[:, :], xsb[t][:, :])
                nc.sync.dma_start(out=out_p[t * P:(t + 1) * P, :],
                                  in_=xsb[t][:, :])
        es_s3.close()
        es_ffw.close()


_NC_CACHE = {}


def _get_nc():
    if "nc" not in _NC_CACHE:
        _NC_CACHE["nc"] = _build_nc()
    return _NC_CACHE["nc"]


def kernel(**inputs):
    nc = _get_nc()
    x = np.asarray(inputs["x"], dtype=np.float32)
    ctx = np.asarray(inputs["context"], dtype=np.float32)
    shared = {k: np.asarray(v, dtype=np.float32) for k, v in inputs.items()
              if k not in ("x", "context")}
    in_maps = []
    for b in range(NCORES):
        m = {"x": np.ascontiguousarray(x[b]),
             "context": np.ascontiguousarray(ctx[b])}
        m.update(shared)
        in_maps.append(m)
    res = run_bass_kernel_spmd(nc, in_maps, list(range(NCORES)))
    out = np.stack([res.results[i]["out"] for i in range(NCORES)], axis=0)
    return out.astype(np.float32)
